# revision 30
# baseline (speedup 1.0000x reference)
"""Trainium2 Bass kernel: SwinV2-style cosine attention w/ CPB relative bias.

Sharding: each of 8 cores owns a 128-wide key/token slice (m-slice) for ALL
batches+heads. Per core: local qkv on its token slice; q normalized/scaled and
AllGathered; rel-bias exp-table gathered on GPSIMD (ap_gather, 16 heads per
index on partition lanes); W^T = exp(S^T)*exp(bias)^T; partial AV + ones-col
row sums; ReduceScatter partials; local softmax-normalize + projection for own
n-slice. Host only slices/permutes/concats.
"""

import contextlib

import numpy as np

import concourse.bass as bass
import concourse.bacc as bacc
import concourse.mybir as mybir
import concourse.tile as tile
from concourse import bass_utils

B = 8
N = 1024
C = 512
H = 16
HD = 32
TABLE = 3969
MS = 128
NCORES = 8
SEQ_SCALE = float(np.log(N))
EPS = 1e-12

F32 = mybir.dt.float32
F32R = mybir.dt.float32r
BF16 = mybir.dt.bfloat16
I16 = mybir.dt.int16
AF = mybir.ActivationFunctionType


def build():
    nc = bacc.Bacc("TRN2", target_bir_lowering=False, debug=False,
                   num_devices=NCORES)

    d_xT = nc.dram_tensor("xT", [C, B * MS], F32R, kind="ExternalInput").ap()
    d_wqk = nc.dram_tensor("wqkT", [C, 2 * C], F32R, kind="ExternalInput").ap()
    d_wv = nc.dram_tensor("wvT", [C, C], F32R, kind="ExternalInput").ap()
    d_qkvb = nc.dram_tensor("qkvb", [3 * C], F32, kind="ExternalInput").ap()
    d_bvv = nc.dram_tensor("bvv", [1, C], F32R, kind="ExternalInput").ap()
    d_pw = nc.dram_tensor("projWT", [C, C], F32, kind="ExternalInput").ap()
    d_pb = nc.dram_tensor("projb", [C], F32, kind="ExternalInput").ap()
    d_ctT = nc.dram_tensor("ctT", [2, TABLE], F32R, kind="ExternalInput").ap()
    d_f1w = nc.dram_tensor("fc1wT", [2, C], F32R, kind="ExternalInput").ap()
    d_f1b = nc.dram_tensor("fc1b", [C], F32, kind="ExternalInput").ap()
    d_f2w = nc.dram_tensor("fc2T", [C, H], F32, kind="ExternalInput").ap()
    d_f2b = nc.dram_tensor("fc2b", [H, 1], F32, kind="ExternalInput").ap()
    d_emb = nc.dram_tensor("embT", [HD, H], F32, kind="ExternalInput").ap()
    d_tmp = nc.dram_tensor("temp", [1, H], F32, kind="ExternalInput").ap()
    d_idx = nc.dram_tensor("idxw", [128, N], I16, kind="ExternalInput").ap()
    d_ones = nc.dram_tensor("ones", [HD, 128], F32R,
                            kind="ExternalInput").ap()
    d_out = nc.dram_tensor("yT", [C, B * MS], F32, kind="ExternalOutput").ap()

    g_qin = nc.dram_tensor("qag_in", [C, B * MS], F32R, kind="Internal").ap()
    g_qout = nc.dram_tensor("qag_out", [NCORES, C, B * MS], F32R,
                            kind="Internal", addr_space="Shared").ap()
    g_rin = nc.dram_tensor("rs_in", [NCORES, B, H, 33, 128], F32,
                           kind="Internal").ap()
    g_rout = nc.dram_tensor("rs_out", [B, H, 33, 128], F32,
                            kind="Internal").ap()
    g_gst = nc.dram_tensor("gst", [4, 128, 4096], BF16,
                           kind="Internal").ap()

    with tile.TileContext(nc) as tc:
        with contextlib.ExitStack() as ctx:
            ctx.enter_context(
                nc.allow_low_precision(reason="f32r attention compute"))
            per = ctx.enter_context(tc.tile_pool(name="per", bufs=1))
            wrk = ctx.enter_context(tc.tile_pool(name="wrk", bufs=3))
            ps = ctx.enter_context(
                tc.tile_pool(name="ps", bufs=2, space="PSUM"))
            dmae = [nc.sync, nc.scalar]

            def mm(out, lhsT, rhs, start=True, stop=True):
                ni = nc.tensor.nop(hint="dep").ins
                ni.ins = [nc.tensor.lower_ap(lhsT), nc.tensor.lower_ap(rhs)]
                nc.tensor.matmul(out, lhsT, rhs, start=start, stop=stop)

            # ------- persistent small loads -------
            xT = []
            for kt in range(4):
                t = per.tile([128, B * MS], F32R, tag=f"xT{kt}",
                             name=f"xT{kt}")
                nc.sync.dma_start(out=t, in_=d_xT[128 * kt:128 * (kt + 1), :])
                xT.append(t)
            qkvb = per.tile([128, 12], F32, tag="qkvb")
            nc.sync.dma_start(
                out=qkvb, in_=d_qkvb.rearrange("(a p) -> p a", p=128))
            pbc = per.tile([128, 4], F32, tag="pbc")
            nc.sync.dma_start(out=pbc,
                              in_=d_pb.rearrange("(a p) -> p a", p=128))
            embT = per.tile([HD, H], F32, tag="embT")
            nc.sync.dma_start(out=embT, in_=d_emb)
            tmpT = per.tile([1, H], F32, tag="tmpT")
            nc.sync.dma_start(out=tmpT, in_=d_tmp)
            idxw = per.tile([128, N], I16, tag="idxw")
            nc.sync.dma_start(out=idxw, in_=d_idx)
            f1w = per.tile([2, C], F32R, tag="f1w")
            nc.sync.dma_start(out=f1w, in_=d_f1w)
            f1b = per.tile([128, 4], F32, tag="f1b")
            nc.sync.dma_start(out=f1b, in_=d_f1b.rearrange("(a p) -> p a",
                                                           p=128))
            f2wf = wrk.tile([128, 4 * H], F32, tag="f2wf", bufs=1)
            nc.sync.dma_start(
                out=f2wf.rearrange("p (a h) -> p a h", a=4),
                in_=d_f2w.rearrange("(a p) h -> p a h", p=128))
            f2w = per.tile([128, 4 * H], BF16, tag="f2w")
            nc.vector.tensor_copy(f2w, f2wf)
            f2b = per.tile([H, 1], F32, tag="f2b")
            nc.sync.dma_start(out=f2b, in_=d_f2b)
            bvv = per.tile([1, C], F32R, tag="bvv")
            nc.sync.dma_start(out=bvv, in_=d_bvv)
            onesT = per.tile([HD, 128], F32R, tag="onesT")
            nc.sync.dma_start(out=onesT, in_=d_ones)
            ones32 = onesT[:, 0:1]

            ones1 = onesT[0:1, :]
            sclT = per.tile([1, H], F32, tag="sclT")
            spe = per.tile([1, H], F32, tag="spe")
            nc.scalar.activation(spe, tmpT, AF.Exp, 0.0, 1.0)
            nc.vector.tensor_scalar_add(spe, spe, 1.0)
            nc.scalar.activation(sclT, spe, AF.Ln, 0.0, 1.0)
            sclR = per.tile([1, H], F32R, tag="sclR")
            nc.vector.tensor_scalar_mul(sclR, sclT, SEQ_SCALE)
            prep = ps.tile([HD, 512], F32, tag="rep")
            mm(prep[:, :H], ones1[:, 0:HD], sclR,
                             start=True, stop=True)
            scl32 = per.tile([HD, H], F32, tag="scl32")
            nc.vector.tensor_copy(scl32, prep[:, :H])

            # ------- CPB MLP -> packed exp table (chunked) -------
            tP = per.tile([128, 2 * TABLE], BF16, tag="tP")
            tPv = tP.rearrange("p (e two) -> p e two", two=2)
            for chi in range(8):
                cw = min(512, TABLE - 512 * chi)
                cwp = cw + (cw % 2)
                ctc = wrk.tile([2, 512], F32R, tag="ctc", bufs=1)
                nc.sync.dma_start(out=ctc[:, :cw],
                                  in_=d_ctT[:, 512 * chi:512 * chi + cw])
                rTc = wrk.tile([128, 4 * 512], BF16, tag="rTc", bufs=1)
                rv = rTc.rearrange("p (a f) -> p a f", a=4)
                for j in range(4):
                    pt = ps.tile([128, 512], F32, tag="s")
                    mm(
                        pt[:, :cwp],
                        f1w[:, 128 * j:128 * (j + 1)],
                        ctc[:, :cwp], start=True, stop=True)
                    nc.scalar.activation(rv[:, j, :cwp], pt[:, :cwp], AF.Relu,
                                         f1b[:, j:j + 1], 1.0)
                pt2 = ps.tile([33, 512], F32, tag="av")
                for j in range(4):
                    mm(pt2[:H, :cwp], f2w[:, H * j:H * (j + 1)],
                       rv[:, j, :cwp], start=(j == 0), stop=(j == 3))
                texc = wrk.tile([H, 512], F32, tag="texc", bufs=1)
                nc.scalar.activation(texc[:, :cw], pt2[:H, :cw], AF.Exp,
                                     f2b, 1.0)
                nc.vector.tensor_copy(
                    tPv[:H, 512 * chi:512 * chi + cw, 0:1],
                    texc[:, :cw].unsqueeze(2))
                nc.vector.tensor_copy(
                    tPv[:H, 512 * chi:512 * chi + cw, 1:2],
                    texc[:, :cw].unsqueeze(2))
            for rep in (16, 32, 64):
                nc.sync.dma_start(out=tP[rep:2 * rep, :], in_=tP[0:rep, :])

            # ------- GPSIMD gather of exp(bias) + DMA rearrange -------
            GT = [per.tile([128, N], BF16, tag=f"GT{h}", name=f"GT{h}")
                  for h in range(H)]
            for quad in range(4):
                gd = wrk.tile([128, 4096], F32, tag="gd", bufs=1)
                nc.gpsimd.ap_gather(
                    gd, tP.bitcast(F32)[:, :TABLE],
                    idxw[:, 256 * quad:256 * (quad + 1)],
                    channels=128, num_elems=TABLE, d=1, num_idxs=4096)
                gc = wrk.tile([128, 4096], BF16, tag="gc", bufs=1,
                              name=f"gc{quad}")
                nc.vector.tensor_copy(
                    gc.unsqueeze(2),
                    gd.bitcast(BF16).rearrange(
                        "p (e two) -> p e two", two=2)[:, :, 0:1])
                nc.sync.dma_start(
                    out=g_gst[quad:quad + 1, :, :].squeeze(0), in_=gc)
                gvd = g_gst[quad:quad + 1, :, :].squeeze(0).rearrange(
                    "p (n m) -> p n m", n=32, m=128)
                for q in range(8):
                    for h in range(H):
                        src = (gvd[16 * q + h:16 * q + h + 1, :, :]
                               .squeeze(0).transpose([1, 0]))
                        dmae[(q * H + h) % 2].dma_start(
                            out=GT[h][:, 128 * q + 32 * quad:
                                      128 * q + 32 * quad + 32],
                            in_=src)

            # ------- qkv projection + normalize (streamed weights) -------
            qnT = [per.tile([128, B * MS], F32R, tag=f"qnT{i}",
                            name=f"qnT{i}") for i in range(4)]
            knT = [per.tile([128, B * MS], F32R, tag=f"knT{i}",
                            name=f"knT{i}") for i in range(4)]

            def proj_co(co):
                dst = wrk.tile([128, B * MS], F32, tag="qkco", bufs=2,
                               name=f"qk{co}")
                for nh in range(2):
                    pt = ps.tile([128, 512], F32, tag="s")
                    for kt in range(4):
                        wq = wrk.tile([128, 128], F32R, tag="wq", bufs=3,
                                      name=f"wq{co}_{nh}_{kt}")
                        nc.sync.dma_start(
                            out=wq, in_=d_wqk[128 * kt:128 * (kt + 1),
                                              128 * co:128 * (co + 1)])
                        mm(
                            pt, wq,
                            xT[kt][:, 512 * nh:512 * (nh + 1)],
                            start=(kt == 0), stop=(kt == 3))
                    nc.vector.tensor_scalar_add(
                        dst[:, 512 * nh:512 * (nh + 1)], pt,
                        qkvb[:, co:co + 1])
                return dst

            def l2recip(src, nm):
                sq = wrk.tile([HD, B * MS], F32R, tag="sq", bufs=1,
                              name=f"sq{nm}")
                nc.vector.tensor_mul(sq, src, src)
                rt = wrk.tile([1, B * MS], F32, tag="rt", bufs=1,
                              name=f"rt{nm}")
                r = wrk.tile([1, B * MS], F32R, tag="rv3", bufs=1,
                             name=f"r{nm}")
                for nh in range(2):
                    pt = ps.tile([1, 512], F32, tag="nrm", name=f"pn{nm}{nh}")
                    mm(
                        pt, ones32,
                        sq[:, 512 * nh:512 * (nh + 1)],
                        start=True, stop=True)
                    nc.scalar.sqrt(rt[:, 512 * nh:512 * (nh + 1)], pt)
                nc.vector.tensor_scalar_max(rt, rt, EPS)
                nc.vector.reciprocal(r, rt)
                return r

            for co in range(4):
                qco = proj_co(co)
                kco = proj_co(4 + co)
                for hh in range(4):
                    h = 4 * co + hh
                    ro = 32 * hh
                    rq = l2recip(qco[ro:ro + HD, :], f"q{h}")
                    for nh in range(2):
                        rep = ps.tile([HD, 512], F32, tag="rep",
                                      name=f"rpq{h}{nh}")
                        mm(rep, ones1[:, 0:HD],
                                         rq[:, 512 * nh:512 * (nh + 1)],
                                         start=True, stop=True)
                        nc.vector.tensor_mul(
                            qnT[co][ro:ro + HD, 512 * nh:512 * (nh + 1)],
                            qco[ro:ro + HD, 512 * nh:512 * (nh + 1)], rep)
                    nc.vector.tensor_scalar(
                        qnT[co][ro:ro + HD, :], qnT[co][ro:ro + HD, :],
                        embT[:, h:h + 1], scl32[:, h:h + 1],
                        mybir.AluOpType.add, mybir.AluOpType.mult)
                    rk = l2recip(kco[ro:ro + HD, :], f"k{h}")
                    for nh in range(2):
                        rep = ps.tile([HD, 512], F32, tag="rep",
                                      name=f"rpk{h}{nh}")
                        mm(rep, ones1[:, 0:HD],
                                         rk[:, 512 * nh:512 * (nh + 1)],
                                         start=True, stop=True)
                        nc.vector.tensor_mul(
                            knT[co][ro:ro + HD, 512 * nh:512 * (nh + 1)],
                            kco[ro:ro + HD, 512 * nh:512 * (nh + 1)], rep)

            # v projection -> vaug per batch
            pbv = ps.tile([128, 512], F32, tag="s")
            mm(pbv, ones1, bvv, start=True, stop=True)
            bvr = per.tile([128, C], F32, tag="bvr")
            nc.vector.tensor_copy(bvr, pbv)
            vaug = [per.tile([128, 33 * H], BF16, tag=f"va{b}",
                             name=f"va{b}") for b in range(B)]
            for b in range(B):
                pt = ps.tile([128, 512], F32, tag="s")
                for kt in range(4):
                    wvt = wrk.tile([128, C], F32R, tag="wvt", bufs=2,
                                   name=f"wv{b}_{kt}")
                    nc.sync.dma_start(out=wvt,
                                      in_=d_wv[128 * kt:128 * (kt + 1), :])
                    mm(
                        pt, xT[kt][:, 128 * b:128 * (b + 1)],
                        wvt, start=(kt == 0), stop=(kt == 3))
                vb = wrk.tile([128, C], F32, tag="vb", bufs=1)
                nc.vector.tensor_add(vb, pt, bvr)
                vv = vaug[b].rearrange("p (h e) -> p h e", e=33)
                nc.vector.tensor_copy(
                    vv[:, :, 0:32], vb.rearrange("p (h e) -> p h e", e=32))
                nc.vector.memset(vv[:, :, 32:33], 1.0)

            # ------- AllGather qn -------
            for co in range(4):
                nc.sync.dma_start(out=g_qin[128 * co:128 * (co + 1), :],
                                  in_=qnT[co])
            nc.gpsimd.collective_compute(
                "AllGather", mybir.AluOpType.bypass,
                replica_groups=[list(range(NCORES))],
                ins=[g_qin], outs=[g_qout])

            # ------- attention loop -------
            for b in range(B):
                for h in range(H):
                    co, ro = divmod(32 * h, 128)
                    qn = wrk.tile([HD, NCORES * MS], F32R, tag="qn", bufs=2,
                                  name=f"qn{b}_{h}")
                    nc.sync.dma_start(
                        out=qn.rearrange("p (c m) -> p c m", c=NCORES),
                        in_=g_qout[:, 128 * co + ro:128 * co + ro + HD,
                                   MS * b:MS * (b + 1)].transpose([1, 0, 2]))
                    ks = wrk.tile([HD, MS], F32R, tag="ks", bufs=3,
                                  name=f"ks{b}_{h}")
                    nc.vector.tensor_copy(
                        ks, knT[co][ro:ro + HD, MS * b:MS * (b + 1)])
                    wt = wrk.tile([128, N], BF16, tag="wt", bufs=2,
                                  name=f"wt{b}_{h}")
                    for nh in range(2):
                        st = ps.tile([128, 512], F32, tag="s",
                                     name=f"st{b}_{h}_{nh}")
                        mm(
                            st, ks,
                            qn[:, 512 * nh:512 * (nh + 1)],
                            start=True, stop=True)
                        w0 = wrk.tile([128, 512], BF16, tag="w0", bufs=2,
                                      name=f"w0{b}_{h}_{nh}")
                        nc.scalar.activation(w0, st, AF.Exp, 0.0, 1.0)
                        nc.vector.tensor_mul(
                            wt[:, 512 * nh:512 * (nh + 1)], w0,
                            GT[h][:, 512 * nh:512 * (nh + 1)])
                    stg = wrk.tile([33, N], F32, tag="stg", bufs=1,
                                   name=f"stg{b}_{h}")
                    vv = vaug[b].rearrange("p (h e) -> p h e", e=33)
                    for nh in range(2):
                        av = ps.tile([33, 512], F32, tag="av",
                                     name=f"av{b}_{h}_{nh}")
                        mm(
                            av, vv[:, h:h + 1, :].squeeze(1),
                            wt[:, 512 * nh:512 * (nh + 1)],
                            start=True, stop=True)
                        nc.vector.tensor_copy(
                            stg[:, 512 * nh:512 * (nh + 1)], av)
                    nc.sync.dma_start(
                        out=g_rin[:, b:b + 1, h:h + 1, :, :].squeeze(2)
                        .squeeze(1).transpose([1, 0, 2]),
                        in_=stg.rearrange("p (r m) -> p r m", r=NCORES))

            # ------- ReduceScatter partials -------
            nc.gpsimd.collective_compute(
                "ReduceScatter", mybir.AluOpType.add,
                replica_groups=[list(range(NCORES))],
                ins=[g_rin], outs=[g_rout])

            # ------- normalize + assemble aoT + projection -------
            aoT = [per.tile([128, B * MS], BF16, tag=f"ao{kt}",
                            name=f"ao{kt}") for kt in range(4)]
            for b in range(B):
                for h in range(H):
                    co, ro = divmod(32 * h, 128)
                    z = wrk.tile([33, MS], F32, tag="z", bufs=2,
                                 name=f"z{b}_{h}")
                    nc.sync.dma_start(
                        out=z, in_=g_rout[b:b + 1, h:h + 1, :, :]
                        .squeeze(1).squeeze(0))
                    rz = wrk.tile([1, MS], F32R, tag="rz", bufs=2,
                                  name=f"rz{b}_{h}")
                    nc.vector.reciprocal(rz, z[32:33, :])
                    repz = ps.tile([HD, MS], F32, tag="rep",
                                   name=f"rpz{b}_{h}")
                    mm(repz, ones1[:, 0:HD], rz,
                                     start=True, stop=True)
                    nc.vector.tensor_mul(
                        aoT[co][ro:ro + HD, MS * b:MS * (b + 1)],
                        z[0:HD, :], repz)
            for co in range(4):
                for nh in range(2):
                    pt = ps.tile([128, 512], F32, tag="s",
                                 name=f"pj{co}_{nh}")
                    for kt in range(4):
                        pwf = wrk.tile([128, 128], F32, tag="pwf", bufs=2,
                                       name=f"pwf{co}_{nh}_{kt}")
                        nc.sync.dma_start(
                            out=pwf, in_=d_pw[128 * kt:128 * (kt + 1),
                                              128 * co:128 * (co + 1)])
                        pwb = wrk.tile([128, 128], BF16, tag="pwb", bufs=2,
                                       name=f"pwb{co}_{nh}_{kt}")
                        nc.vector.tensor_copy(pwb, pwf)
                        mm(
                            pt, pwb, aoT[kt][:, 512 * nh:512 * (nh + 1)],
                            start=(kt == 0), stop=(kt == 3))
                    yt = wrk.tile([128, 512], F32, tag="yt", bufs=1,
                                  name=f"yt{co}_{nh}")
                    nc.vector.tensor_scalar_add(yt, pt, pbc[:, co:co + 1])
                    nc.sync.dma_start(
                        out=d_out[128 * co:128 * (co + 1),
                                  512 * nh:512 * (nh + 1)],
                        in_=yt)
    nc.compile()
    return nc


_NC_CACHE = None


def _get_nc():
    global _NC_CACHE
    if _NC_CACHE is None:
        _NC_CACHE = build()
    return _NC_CACHE


def kernel(**inputs):
    x = np.asarray(inputs["x"], np.float32)
    idx = np.asarray(inputs["relative_pos_index"]).astype(np.int64)
    ct = np.asarray(inputs["relative_coords_table"], np.float32)
    qkv_w = np.asarray(inputs["qkv_w"], np.float32)
    qkv_b = np.asarray(inputs["qkv_b"], np.float32)
    proj_w = np.asarray(inputs["proj_w"], np.float32)
    proj_b = np.asarray(inputs["proj_b"], np.float32)
    temp = np.asarray(inputs["temperature"], np.float32)
    emb = np.asarray(inputs["query_embedding"], np.float32)
    f1w = np.asarray(inputs["cpb_fc1_w"], np.float32)
    f1b = np.asarray(inputs["cpb_fc1_b"], np.float32)
    f2w = np.asarray(inputs["cpb_fc2_w"], np.float32)
    f2b = np.asarray(inputs["cpb_fc2_b"], np.float32)

    nc = _get_nc()
    com = {
        "wqkT": np.ascontiguousarray(qkv_w[:2 * C].T),
        "wvT": np.ascontiguousarray(qkv_w[2 * C:].T),
        "qkvb": qkv_b,
        "bvv": qkv_b[2 * C:].reshape(1, C),
        "projWT": np.ascontiguousarray(proj_w.T),
        "projb": proj_b,
        "ctT": np.ascontiguousarray(ct.T),
        "fc1wT": np.ascontiguousarray(f1w.T),
        "fc1b": f1b,
        "fc2T": np.ascontiguousarray(f2w.T),
        "fc2b": f2b.reshape(H, 1),
        "embT": np.ascontiguousarray(emb.reshape(H, HD).T),
        "temp": temp.reshape(1, H),
        "ones": np.ones((HD, 128), np.float32),
    }
    in_maps = []
    for c in range(NCORES):
        m = dict(com)
        xs = x[:, MS * c:MS * (c + 1), :].reshape(B * MS, C)
        m["xT"] = np.ascontiguousarray(xs.T)
        A = idx[:, MS * c:MS * (c + 1)].astype(np.int16)  # [1024 n, 128 m]
        L = A.reshape(8, 128, 128).reshape(8, 16384)  # j = nloc*128+mloc
        W = L.reshape(8, 1024, 16).transpose(0, 2, 1)  # [q, lane, col]
        m["idxw"] = np.ascontiguousarray(W.reshape(128, N))
        in_maps.append(m)

    res = bass_utils.run_bass_kernel_spmd(nc, in_maps,
                                          core_ids=list(range(NCORES)))
    out = np.empty((B, N, C), np.float32)
    for c in range(NCORES):
        yT = res.results[c]["yT"]
        out[:, MS * c:MS * (c + 1), :] = (
            yT.reshape(C, B, MS).transpose(1, 2, 0))
    return out


if __name__ == "__main__":
    build()
    print("build OK")
